# revision 42
# baseline (speedup 1.0000x reference)
"""Trainium2 Bass kernel for nn_HFMiMoV2DecoderLayer (attention + MoE decoder layer).

Strategy (8 NeuronCores):
  Launch 1 — tensor-parallel attention: each core owns 2 of 16 heads (and the
    matching GQA KV head). Host pre-transposes x to x^T [H, T] and precomputes
    rmsnorm scales r1 (exact fp32, folded into Q/V evictions and the exp
    scale), rope tables in transposed layout, sink exp, and the causal band
    mask. Phase A: QKV weight-stationary matmuls (512-col moving fp32r,
    priority-interleaved DMA so compute starts with the first hc chunk), rope
    via a signed-permutation PE matmul + gpsimd/vector elementwise. Fused
    phase B+C per 512-token query block: k-major flash with sink-softmax
    (denominator via full-ones matmul = pre-broadcast column sums,
    reciprocal_approx_fast normalize), then the Wo partial for that block so
    the [T, H] output DMA streams during flash.
  Host    — h1 = x + sum(partials); exact MoE routing (numpy, mirrors the
    reference); builds per-expert gathered activation matrices (transposed).
  Launch 2 — expert-parallel MoE FF in bf16 (routing already fixed on host, so
    only output precision matters; bf16 end-to-end error ~1e-3 << 2e-2 gate):
    weight-stationary gate/up with hc-interleaved priority DMA, silu*up*
    combine (host-broadcast combine weights), down-proj with hgu-stationary.
  Host    — scatter-add contributions into h1.

The h1/routing path stays fp32(r) end-to-end: min routing margin for this
layer's data is ~3e-5, so any bf16 before the gate would flip top-k choices.
"""
import sys
import types

import numpy as np
import ml_dtypes


def _install_ntff_hook():
    """bass_utils needs antenv.axon_hooks for NTFF tracing under axon; the
    image's antenv lacks that submodule. Inject a shim wired to the ctypes
    hook from trn_agent_boot (no-op if anything is missing)."""
    if "antenv.axon_hooks" in sys.modules:
        return
    try:
        from trn_agent_boot.trn_boot import _ntff_profile_via_ctypes

        hook = _ntff_profile_via_ctypes("/opt/axon/libaxon_pjrt.so")
    except Exception:
        hook = None
    mod = types.ModuleType("antenv.axon_hooks")
    mod._hook = hook
    mod.set_axon_ntff_profile_hook = lambda h: setattr(mod, "_hook", h)
    mod.get_axon_ntff_profile_hook = lambda: mod._hook
    sys.modules["antenv.axon_hooks"] = mod


_install_ntff_hook()

import concourse.bass as bass
import concourse.mybir as mybir
import concourse.tile as tile
from concourse import bacc
from concourse.bass_utils import run_bass_kernel_spmd
from concourse.masks import make_identity

F32 = mybir.dt.float32
F32R = mybir.dt.float32r
BF16 = mybir.dt.bfloat16

N_CORES = 8
T = 2048          # tokens
H = 2048          # hidden
P = 128
HCH = H // P      # 16 hidden chunks
HD = 128          # head dim
NHC = 2           # heads per core
RD = 64           # rope dims
RH = 32
FF = 512          # moe intermediate
E = 16
EPC = 2           # experts per core
SCALE = HD ** -0.5
EPS = 1e-6
ROUTE_SCALE = 2.5
G, TG, TK = 4, 2, 4

QG = 512          # query-group width for attention
NQG = T // QG     # 4
NQT = T // 512    # 4 token quarters in phase A


def _r32(ap):
    return ap.bitcast(F32R)


def _mk_nc():
    return bacc.Bacc("TRN2", target_bir_lowering=False, debug=False,
                     num_devices=N_CORES)


# --------------------------------------------------------------------------
# Launch 1: attention (2 heads per core)
# --------------------------------------------------------------------------

def build_attn():
    nc = _mk_nc()
    xt = nc.dram_tensor("xt", [H, T], F32R, kind="ExternalInput")
    wqkv = nc.dram_tensor("wqkv", [H, 4 * HD], F32R, kind="ExternalInput")
    wo = nc.dram_tensor("wo", [NHC * HD, H], F32R, kind="ExternalInput")
    cost = nc.dram_tensor("cost", [RD, T], F32, kind="ExternalInput")
    sintf = nc.dram_tensor("sintf", [RD, T], F32, kind="ExternalInput")
    r1bc = nc.dram_tensor("r1bc", [P, T], F32, kind="ExternalInput")
    r1ks = nc.dram_tensor("r1ks", [P, HCH], F32, kind="ExternalInput")
    sinke = nc.dram_tensor("sinke", [P, NHC], F32, kind="ExternalInput")
    maskt = nc.dram_tensor("maskt", [P, P], F32R, kind="ExternalInput")
    rotm = nc.dram_tensor("rotm", [RD, RD], F32R, kind="ExternalInput")
    partial = nc.dram_tensor("partial", [T, H], F32, kind="ExternalOutput")

    pt_out = partial.rearrange("(tc p) h -> tc p h", p=P)
    xtr = xt.rearrange("(hc p) t -> p hc t", p=P)

    with tile.TileContext(nc) as tc:
        with (
            tc.tile_pool(name="persist", bufs=1) as pers,
            tc.tile_pool(name="const", bufs=1) as constp,
        ):
            # allocate persistent tiles up front; DMAs are issued in priority
            # order (QKV weights + first x^T quarter first) below
            wo_s = pers.tile([P, NHC, H], F32R)
            cos_s = pers.tile([RD, T], F32)
            sin_s = pers.tile([RD, T], F32)
            r1bc_s = pers.tile([P, T], F32)
            r1ks_s = pers.tile([P, HCH], F32)
            sinke_s = pers.tile([P, NHC], F32)
            mask_s = pers.tile([P, P], F32R)
            rotm_s = pers.tile([RD, RD], F32R)

            ident0 = constp.tile([P, P], F32)
            make_identity(nc, ident0[:])
            ident = constp.tile([P, P], F32R)
            nc.vector.tensor_copy(ident[:], ident0[:])
            ones0 = constp.tile([P, P], F32)
            nc.vector.memset(ones0[:], 1.0)
            ones_f = constp.tile([P, P], F32R)
            nc.vector.tensor_copy(ones_f[:], ones0[:])

            qt_s = pers.tile([P, NHC, T], F32R)      # Q^T·r1q  [hd, h, tok]
            kt_s = pers.tile([P, T], F32R)           # K^T      [hd, tok]
            v_s = pers.tile([P, HCH, HD], F32R)      # V·r1     [tok, tc, hd]
            ot_s = pers.tile([P, NHC, T], F32R)      # O^T      [hd, h, tok]

            # ---------------- phase A: QKV (weight-stationary) + rope ------
            with (
                tc.tile_pool(name="xtp", bufs=2) as xtp,
                tc.tile_pool(name="wqkvp", bufs=1) as wqkvp,
                tc.tile_pool(name="vt", bufs=2) as vtp,
                tc.tile_pool(name="rop", bufs=2) as ropp,
                tc.tile_pool(name="psA", bufs=1, space="PSUM") as psA,
                tc.tile_pool(name="psT", bufs=2, space="PSUM") as psT,
            ):
                wqkv_s = wqkvp.tile([P, HCH, 4 * HD], F32R, tag="wqkv")
                wqkv_r = wqkv.rearrange("(hc p) n -> p hc n", p=P)

                for q in range(NQT):
                    sl = slice(q * 512, (q + 1) * 512)
                    xt_i = xtp.tile([P, HCH, 512], F32R, tag="xt")
                    # interleave weight/x chunks so compute starts immediately
                    for hc in range(HCH):
                        if q == 0:
                            nc.sync.dma_start(wqkv_s[:, hc, :],
                                              wqkv_r[:, hc, :])
                        nc.sync.dma_start(xt_i[:, hc, :], xtr[:, hc, sl])
                    if q == 0:
                        # lower-priority persistent inputs (needed mid-phase-A
                        # at the earliest; wo only in phase C)
                        nc.sync.dma_start(r1bc_s[:], r1bc[:])
                        nc.sync.dma_start(cos_s[:], cost[:])
                        nc.sync.dma_start(sin_s[:], sintf[:])
                        nc.sync.dma_start(r1ks_s[:], r1ks[:])
                        nc.sync.dma_start(sinke_s[:], sinke[:])
                        nc.sync.dma_start(mask_s[:], maskt[:])
                        nc.sync.dma_start(rotm_s[:], rotm[:])
                    if q == NQT - 1:
                        # wo only needed in phase B+C; lowest priority
                        nc.sync.dma_start(
                            wo_s[:], wo.rearrange("(h p) n -> p h n", p=P))
                    # 4 column groups: q-head0, q-head1, K, V
                    ps_q0 = psA.tile([P, 512], F32, tag="ps0")
                    ps_q1 = psA.tile([P, 512], F32, tag="ps1")
                    ps_k = psA.tile([P, 512], F32, tag="ps2")
                    ps_v = psA.tile([P, 512], F32, tag="ps3")
                    pss = [ps_q0, ps_q1, ps_k, ps_v]
                    for hc in range(HCH):
                        for cg in range(4):
                            nc.tensor.matmul(
                                pss[cg][:],
                                _r32(wqkv_s[:, hc, cg * P:(cg + 1) * P]),
                                _r32(xt_i[:, hc, :]),
                                start=(hc == 0), stop=(hc == HCH - 1))
                    for cg in range(4):
                        ps = pss[cg]
                        if cg < NHC:     # Q head: scale by r1[q]
                            nc.vector.tensor_mul(qt_s[:, cg, sl], ps[:],
                                                 r1bc_s[:, sl])
                        elif cg == NHC:  # K: plain (r1k folded into exp scale)
                            nc.scalar.activation(
                                kt_s[:, sl], ps[:],
                                mybir.ActivationFunctionType.Copy)
                        else:            # V: scale by r1, then transpose
                            vt_tmp = vtp.tile([P, 512], F32R, tag="vt")
                            nc.vector.tensor_mul(vt_tmp[:], ps[:],
                                                 r1bc_s[:, sl])
                            for tc4 in range(4):
                                pst = psT.tile([P, P], F32R, tag="pst")
                                nc.tensor.transpose(
                                    pst[:], vt_tmp[:, tc4 * P:(tc4 + 1) * P],
                                    ident[:])
                                nc.vector.tensor_copy(
                                    v_s[:, q * 4 + tc4, :], pst[:])
                    # rope for this quarter's columns (q0, q1, k):
                    # rotate_half via PE permutation matmul, rest elementwise
                    for tgt in (qt_s[:, 0, sl], qt_s[:, 1, sl], kt_s[:, sl]):
                        ps_r = psT.tile([RD, 512], F32, tag="psr")
                        nc.tensor.matmul(ps_r[:], _r32(rotm_s[:]),
                                         _r32(tgt[0:RD, :]),
                                         start=True, stop=True)
                        ta = ropp.tile([RD, 512], F32, tag="ra")
                        tb = ropp.tile([RD, 512], F32, tag="rb")
                        nc.gpsimd.tensor_mul(ta[:], tgt[0:RD, :],
                                             cos_s[:, sl])
                        nc.vector.tensor_mul(tb[:], ps_r[:], sin_s[:, sl])
                        nc.gpsimd.tensor_add(tgt[0:RD, :], ta[:], tb[:])

            # ---------- fused phase B+C: flash + Wo per query block ----------
            with (
                tc.tile_pool(name="psS", bufs=3, space="PSUM") as psS,
                tc.tile_pool(name="psO", bufs=2, space="PSUM") as psO,
                tc.tile_pool(name="psD", bufs=1, space="PSUM") as psD,
                tc.tile_pool(name="psW", bufs=2, space="PSUM") as psW,
                tc.tile_pool(name="ptp", bufs=3) as ptp,
                tc.tile_pool(name="den", bufs=2) as denp,
                tc.tile_pool(name="outc", bufs=3) as outc,
            ):
                for qg in range(NQG):
                    nkt = 4 * (qg + 1)
                    for h in range(NHC):
                        ps_d = psD.tile([P, QG], F32, tag="psd")
                        q_rhs = _r32(qt_s[:, h, qg * QG:(qg + 1) * QG])
                        ps_o = psO.tile([P, QG], F32, tag="pso")
                        for kt in range(nkt):
                            d = kt - 4 * qg
                            off = max(d, 0) * P
                            ps_s = psS.tile([P, QG], F32, tag="pss")
                            nc.tensor.matmul(ps_s[:, off:],
                                             _r32(kt_s[:, kt * P:(kt + 1) * P]),
                                             q_rhs[:, off:],
                                             start=True, stop=True)
                            p_t = ptp.tile([P, QG], F32R, tag="pt")
                            nc.scalar.activation(p_t[:, off:], ps_s[:, off:],
                                                 mybir.ActivationFunctionType.Exp,
                                                 scale=r1ks_s[:, kt:kt + 1])
                            if d >= 0:  # diagonal 128-col band gets the mask
                                nc.vector.tensor_mul(
                                    p_t[:, off:off + P], p_t[:, off:off + P],
                                    mask_s[:])
                            nc.tensor.matmul(ps_o[:, off:], _r32(v_s[:, kt, :]),
                                             _r32(p_t[:, off:]),
                                             start=(kt == 0),
                                             stop=(kt == nkt - 1))
                            # den pre-broadcast: every row of ones.T @ p is
                            # the column sum
                            nc.tensor.matmul(ps_d[:, off:],
                                             _r32(ones_f[:]),
                                             _r32(p_t[:, off:]),
                                             start=(kt == 0),
                                             stop=(kt == nkt - 1))
                        # normalize: ot = O' * approx(1/(den + sink))
                        dsb = denp.tile([P, QG], F32, tag="dsb")
                        nc.vector.tensor_scalar(dsb[:], ps_d[:],
                                                sinke_s[:, h:h + 1], None,
                                                mybir.AluOpType.add)
                        rec = denp.tile([P, QG], F32, tag="rec")
                        nc.vector.reciprocal_approx_fast(rec[:], dsb[:])
                        nc.vector.tensor_mul(ot_s[:, h, qg * QG:(qg + 1) * QG],
                                             ps_o[:], rec[:])
                    # ---- Wo for this block's 4 token chunks; DMA streams out
                    for ti4 in range(4):
                        ti = qg * 4 + ti4
                        out_sb = outc.tile([P, H], F32, tag="osb")
                        for nt in range(H // 512):
                            ps_w = psW.tile([P, 512], F32, tag="psw")
                            for h in range(NHC):
                                nc.tensor.matmul(
                                    ps_w[:],
                                    _r32(ot_s[:, h, ti * P:(ti + 1) * P]),
                                    _r32(wo_s[:, h, nt * 512:(nt + 1) * 512]),
                                    start=(h == 0), stop=(h == NHC - 1))
                            osl = slice(nt * 512, (nt + 1) * 512)
                            if nt % 2 == 0:
                                nc.scalar.activation(
                                    out_sb[:, osl], ps_w[:],
                                    mybir.ActivationFunctionType.Copy)
                            else:
                                nc.vector.tensor_copy(out_sb[:, osl], ps_w[:])
                        nc.sync.dma_start(pt_out[ti], out_sb[:])
    nc.finalize()
    return nc


# --------------------------------------------------------------------------
# Launch 2: MoE expert FF in bf16 (2 experts per core, capacity C tokens)
# --------------------------------------------------------------------------

def build_moe(c_cap):
    nc = _mk_nc()
    xgt = nc.dram_tensor("xgt", [EPC, H, c_cap], BF16, kind="ExternalInput")
    wrow = nc.dram_tensor("wrow", [EPC, P, c_cap], F32R, kind="ExternalInput")
    weg = nc.dram_tensor("weg", [EPC, H, FF], BF16, kind="ExternalInput")
    weu = nc.dram_tensor("weu", [EPC, H, FF], BF16, kind="ExternalInput")
    wed = nc.dram_tensor("wed", [EPC, FF, H], BF16, kind="ExternalInput")
    contrib = nc.dram_tensor("contrib", [EPC * c_cap, H], BF16,
                             kind="ExternalOutput")
    co = contrib.rearrange("(ec tc p) h -> ec tc p h", p=P, ec=EPC)

    # column chunks of c_cap, each <= 512 (psum bank) and bank-aligned
    ccs = []
    o = 0
    while o < c_cap:
        n = min(512, c_cap - o)
        ccs.append((o, n))
        o += n
    ffc_n = FF // P  # 4

    with tile.TileContext(nc) as tc:
        with (
            tc.tile_pool(name="const", bufs=1) as constp,
            tc.tile_pool(name="xg", bufs=2) as xgp,
            tc.tile_pool(name="wgu", bufs=2) as wgup,
            tc.tile_pool(name="wd", bufs=2) as wdp,
            tc.tile_pool(name="hgu", bufs=2) as hgup,
            tc.tile_pool(name="act", bufs=3) as actp,
            tc.tile_pool(name="wr", bufs=2) as wrp,
            tc.tile_pool(name="outp", bufs=3) as outp,
        ):
            # DMA issue order = priority: expert-0 g/u inputs interleaved at
            # hc granularity first, then expert-1, then down-proj weights
            xgs, wgs, wus, wds, wbcs, hgus = [], [], [], [], [], []
            xgr = [xgt[e].rearrange("(hc p) c -> p hc c", p=P)
                   for e in range(EPC)]
            wgr = [weg[e].rearrange("(hc p) f -> p hc f", p=P)
                   for e in range(EPC)]
            wur = [weu[e].rearrange("(hc p) f -> p hc f", p=P)
                   for e in range(EPC)]
            for e in range(EPC):
                xg_t = xgp.tile([P, HCH, c_cap], BF16, tag="xg")
                wg_t = wgup.tile([P, HCH, FF], BF16, tag="wg")
                wu_t = wgup.tile([P, HCH, FF], BF16, tag="wu")
                wd_t = wdp.tile([P, ffc_n, H], BF16, tag="wd")
                xgs.append(xg_t)
                wgs.append(wg_t)
                wus.append(wu_t)
                wds.append(wd_t)
            for e in range(EPC):
                wbc = wrp.tile([P, c_cap], F32, tag="wbc")
                nc.sync.dma_start(wbc[:], wrow[e].bitcast(F32))
                wbcs.append(wbc)
            for e in range(EPC):
                for hc in range(HCH):
                    nc.sync.dma_start(xgs[e][:, hc, :], xgr[e][:, hc, :])
                    nc.sync.dma_start(wgs[e][:, hc, :], wgr[e][:, hc, :])
                    nc.sync.dma_start(wus[e][:, hc, :], wur[e][:, hc, :])
            for e in range(EPC):
                nc.sync.dma_start(
                    wds[e][:], wed[e].rearrange("(fc p) h -> p fc h", p=P))

            with (
                tc.tile_pool(name="psG", bufs=2, space="PSUM") as psG,
                tc.tile_pool(name="psU", bufs=2, space="PSUM") as psU,
            ):
                for e in range(EPC):
                    wg_s, wu_s = wgs[e], wus[e]
                    xg_s, wbc = xgs[e], wbcs[e]
                    hgu = hgup.tile([P, ffc_n, c_cap], BF16, tag="hgu")
                    hgus.append(hgu)
                    for fc in range(ffc_n):
                        ps_g = psG.tile([P, c_cap], F32, tag="psg")
                        ps_u = psU.tile([P, c_cap], F32, tag="psu")
                        fsl = slice(fc * P, (fc + 1) * P)
                        for hc in range(HCH):
                            for o, n in ccs:
                                nc.tensor.matmul(ps_g[:, o:o + n],
                                                 wg_s[:, hc, fsl],
                                                 xg_s[:, hc, o:o + n],
                                                 start=(hc == 0),
                                                 stop=(hc == HCH - 1))
                            for o, n in ccs:
                                nc.tensor.matmul(ps_u[:, o:o + n],
                                                 wu_s[:, hc, fsl],
                                                 xg_s[:, hc, o:o + n],
                                                 start=(hc == 0),
                                                 stop=(hc == HCH - 1))
                        sg = actp.tile([P, c_cap], F32, tag="sg")
                        nc.scalar.activation(sg[:], ps_g[:],
                                             mybir.ActivationFunctionType.Silu)
                        uw = actp.tile([P, c_cap], F32, tag="uw")
                        nc.vector.tensor_mul(uw[:], ps_u[:], wbc[:])
                        nc.vector.tensor_mul(hgu[:, fc, :], sg[:], uw[:])

            # down projection: contrib[tok, H]
            with tc.tile_pool(name="psC", bufs=2, space="PSUM") as psC:
                for e in range(EPC):
                    wd_s, hgu = wds[e], hgus[e]
                    for ti in range(c_cap // P):
                        ps_c = psC.tile([P, H], F32, tag="psc")
                        for fc in range(ffc_n):
                            for nt in range(H // 512):
                                nc.tensor.matmul(
                                    ps_c[:, nt * 512:(nt + 1) * 512],
                                    hgu[:, fc, ti * P:(ti + 1) * P],
                                    wd_s[:, fc, nt * 512:(nt + 1) * 512],
                                    start=(fc == 0), stop=(fc == ffc_n - 1))
                        out_sb = outp.tile([P, H], BF16, tag="osb")
                        nc.scalar.activation(
                            out_sb[:, 0:1024], ps_c[:, 0:1024],
                            mybir.ActivationFunctionType.Copy)
                        nc.vector.tensor_copy(out_sb[:, 1024:2048],
                                              ps_c[:, 1024:2048])
                        nc.sync.dma_start(co[e, ti], out_sb[:])

    nc.finalize()
    return nc


# --------------------------------------------------------------------------
# Host-side routing (numpy mirror of the reference MoE gate)
# --------------------------------------------------------------------------

def _routing(h1, ln2_w, gate_w, gate_bias):
    var = np.mean(h1 * h1, axis=-1, keepdims=True)
    xf = (ln2_w * (h1 / np.sqrt(var + EPS))).astype(np.float32)
    logits = xf @ gate_w.T
    s = 1.0 / (1.0 + np.exp(-logits))
    sfc = s + gate_bias[None]
    n = sfc.shape[0]
    gview = sfc.reshape(n, G, E // G)
    gsort = np.sort(gview, axis=-1)
    group_scores = gsort[..., -1] + gsort[..., -2]
    gidx = np.argsort(-group_scores, kind="stable", axis=-1)[:, :TG]
    gmask = np.zeros((n, G), np.bool_)
    np.put_along_axis(gmask, gidx, True, axis=1)
    smask = np.repeat(gmask, E // G, axis=1)
    tmp = np.where(smask, sfc, -np.inf)
    tidx = np.argsort(-tmp, kind="stable", axis=-1)[:, :TK]
    tw = np.take_along_axis(s, tidx, axis=1)
    tw = tw / (tw.sum(-1, keepdims=True) + 1e-20)
    tw = tw * ROUTE_SCALE
    cw = np.zeros((n, E), np.float32)
    np.put_along_axis(cw, tidx, tw.astype(np.float32), axis=1)
    return xf, cw


# --------------------------------------------------------------------------
# Entry point
# --------------------------------------------------------------------------

_NC_CACHE = {}


def _get_nc(key, builder, *args):
    if key not in _NC_CACHE:
        _NC_CACHE[key] = builder(*args)
    return _NC_CACHE[key]


def kernel(hidden_states, cos, sin, ln1_w, ln2_w, Wq, Wk, Wv, Wo,
           sink_bias, gate_w, gate_bias, Weg, Weu, Wed, _profile=None):
    hidden_states, cos, sin, ln1_w, ln2_w = map(
        np.asarray, (hidden_states, cos, sin, ln1_w, ln2_w))
    Wq, Wk, Wv, Wo, sink_bias = map(np.asarray, (Wq, Wk, Wv, Wo, sink_bias))
    gate_w, gate_bias, Weg, Weu, Wed = map(
        np.asarray, (gate_w, gate_bias, Weg, Weu, Wed))
    b, s, _ = hidden_states.shape
    x = np.ascontiguousarray(hidden_states.reshape(T, H), dtype=np.float32)
    cosb = cos.reshape(T, RD).astype(np.float32)
    sinb = sin.reshape(T, RD).astype(np.float32)

    # host-side rmsnorm scales (exact fp32, matches reference math)
    var = np.mean(x * x, axis=-1)                       # [T]
    r1 = (1.0 / np.sqrt(var + EPS)).astype(np.float32)  # [T]
    xtT = np.ascontiguousarray(x.T)                     # [H, T]

    # rope tables, transposed; rotate_half done on device via rotm matmul
    cost = np.ascontiguousarray(cosb.T)                       # [RD, T]
    sintf = np.ascontiguousarray(sinb.T)                      # [RD, T]
    # lhsT for out = rot_half(q): out[m] = -q[m+32] (m<32), q[m-32] (m>=32)
    rotm = np.zeros((RD, RD), np.float32)
    for m in range(RH):
        rotm[m + RH, m] = -1.0
        rotm[m, m + RH] = 1.0

    r1bc = np.ascontiguousarray(np.broadcast_to(r1[None], (P, T)))
    r1ks = np.ascontiguousarray((r1.reshape(HCH, P).T * SCALE))  # [P, HCH]

    # causal mask for the in-band 128 columns of a diagonal block
    kp = np.arange(P)[:, None]
    qf = np.arange(P)[None, :]
    maskt = (qf >= kp).astype(np.float32)

    # fold ln1 into the QKV weights
    wq_f = (ln1_w[:, None] * Wq).astype(np.float32)
    wk_f = (ln1_w[:, None] * Wk).astype(np.float32)
    wv_f = (ln1_w[:, None] * Wv).astype(np.float32)

    in_maps = []
    for c in range(N_CORES):
        h0 = NHC * c
        g0 = h0 // (16 // 4)  # kv head
        in_maps.append({
            "xt": xtT,
            "wqkv": np.ascontiguousarray(np.concatenate(
                [wq_f[:, h0 * HD:(h0 + NHC) * HD],
                 wk_f[:, g0 * HD:(g0 + 1) * HD],
                 wv_f[:, g0 * HD:(g0 + 1) * HD]], axis=1)),
            "wo": np.ascontiguousarray(Wo[h0 * HD:(h0 + NHC) * HD, :]),
            "cost": cost,
            "sintf": sintf,
            "r1bc": r1bc,
            "r1ks": r1ks,
            "sinke": np.ascontiguousarray(np.broadcast_to(
                np.exp(sink_bias[h0:h0 + NHC]).astype(np.float32)[None],
                (P, NHC))),
            "maskt": maskt,
            "rotm": rotm,
        })

    nc1 = _get_nc("attn", build_attn)
    res1 = run_bass_kernel_spmd(nc1, in_maps, core_ids=list(range(N_CORES)),
                                trace=_profile is not None)
    h1 = x.copy()
    for c in range(N_CORES):
        h1 += res1.results[c]["partial"]

    xf, cw = _routing(h1, np.asarray(ln2_w), np.asarray(gate_w),
                      np.asarray(gate_bias))

    idxs = [np.nonzero(cw[:, e] > 0)[0] for e in range(E)]
    maxc = max(len(ix) for ix in idxs)
    c_cap = max(512, -(-maxc // P) * P)

    in_maps2 = []
    for c in range(N_CORES):
        xg = np.zeros((EPC, H, c_cap), ml_dtypes.bfloat16)
        wr = np.zeros((EPC, P, c_cap), np.float32)
        for j in range(EPC):
            e = EPC * c + j
            ix = idxs[e]
            xg[j, :, :len(ix)] = xf[ix].T.astype(ml_dtypes.bfloat16)
            wr[j, :, :len(ix)] = cw[ix, e][None]
        in_maps2.append({
            "xgt": xg,
            "wrow": wr,
            "weg": Weg[EPC * c:EPC * (c + 1)].astype(ml_dtypes.bfloat16),
            "weu": Weu[EPC * c:EPC * (c + 1)].astype(ml_dtypes.bfloat16),
            "wed": Wed[EPC * c:EPC * (c + 1)].astype(ml_dtypes.bfloat16),
        })

    nc2 = _get_nc(("moe", c_cap), build_moe, c_cap)
    res2 = run_bass_kernel_spmd(nc2, in_maps2, core_ids=list(range(N_CORES)),
                                trace=_profile is not None)

    out = h1
    for c in range(N_CORES):
        cb = res2.results[c]["contrib"].astype(np.float32).reshape(
            EPC, c_cap, H)
        for j in range(EPC):
            e = EPC * c + j
            ix = idxs[e]
            out[ix] += cb[j, :len(ix)]

    if _profile is not None:
        _profile["attn_ns"] = res1.exec_time_ns
        _profile["moe_ns"] = res2.exec_time_ns
        _profile["res1"] = res1
        _profile["res2"] = res2

    return out.reshape(b, s, H)
